# revision 12
# baseline (speedup 1.0000x reference)
"""Trainium2 Bass kernel for nn_DenseRecurrentConsciousnessNet.

Computation (B=65536, H=512, S=256, D=64):
    read_weights  = softmax(query @ W_read + b_read)          [B, S]
    read_content  = read_weights @ memory                     [B, D]   (output)
    write_weights = softmax(query @ W_write + b_write)        [B, S]
    w_mean        = write_weights.mean(0)                     [S]
    c_mean        = (content @ W_content + b_content).mean(0) [D]
    alpha         = where(w_mean > 0.01, w_mean * sigmoid(0.1 * age), 0)
    new_memory    = (1 - alpha[:, None]) * memory + alpha[:, None] * c_mean
    new_age       = age + (w_mean > 0.01)                     (outputs)

Sharding: data-parallel over batch across 8 cores.  Each core computes its
read_content shard plus two tiny partial reductions: sum_b(write_weights)
[S] and sum_b(content) [H].  The host sums the 8 partials (the "all-reduce
mean" of the hint) and applies the O(S*D) EMA update.

Per-core device kernel (b_shard = 8192, 64 row tiles of 128):
  - PE transposes each query tile (contraction over H needs H on partitions),
    then one fused matmul against [W_read | W_write] -> logits [128, 512].
  - Softmax without max-subtraction (logits are provably in [-6, 6] for this
    problem scale; exp stays far from fp32 overflow): ACT exp with accum_out
    giving the row sums for free.
  - read: PE-transpose exp_r, matmul against memory, scale rows by 1/rowsum.
  - write: matmul with stationary (1/rowsum_w) [128,1] contracts the batch
    partition dim -> per-tile sum_b(write_weights) [1, 256], accumulated.
  - content: DVE-accumulate tiles, one final ones-vector matmul contracts
    the partition dim -> sum_b(content) [1, 512].
"""

import os
import time
from contextlib import ExitStack

import numpy as np
import ml_dtypes

import concourse.bass as bass
import concourse.bacc as bacc
import concourse.tile as tile
from concourse import mybir
from concourse.bass_utils import run_bass_kernel_spmd
from concourse.masks import make_identity

N_CORES = 8
B, H, S, D = 65536, 512, 256, 64
P = 128

# Precision scheme for the PE input paths.  False: fp32 data flowing through
# the PE as float32r (full-rate for moving dim >= 256).  True: query/weights/
# memory and the exp_r read path are bf16 (transposes and the D=64 matmul run
# at 1 cycle/row instead of 1.5-4).  PSUM accumulation is fp32 either way.
PE_BF16 = True

F32 = mybir.dt.float32
F32R = mybir.dt.float32r
BF16 = mybir.dt.bfloat16

# Stash of the last hardware run, for the local test harness.
LAST_RESULTS = None
LAST_WALL_NS = None


def build_module(b_shard: int, slab_tiles: int, use_bias: bool):
    """Build and compile the per-core Bass module (SPMD: same program on
    every core, per-core data comes from in_maps)."""
    assert b_shard % (P * slab_tiles) == 0
    n_slabs = b_shard // (P * slab_tiles)
    n_tiles = b_shard // P

    pe_dt = BF16 if PE_BF16 else F32R

    nc = bacc.Bacc(
        "TRN2",
        target_bir_lowering=False,
        debug=False,
        num_devices=N_CORES,
    )

    q_in = nc.dram_tensor("q_in", [b_shard, H], F32, kind="ExternalInput").ap()
    c_in = nc.dram_tensor("c_in", [b_shard, H], F32, kind="ExternalInput").ap()
    wrw_in = nc.dram_tensor("wrw_in", [H, 2 * S], pe_dt, kind="ExternalInput").ap()
    mem_in = nc.dram_tensor("mem_in", [S, D], pe_dt, kind="ExternalInput").ap()
    if use_bias:
        brw_in = nc.dram_tensor(
            "brw_in", [1, 2 * S], pe_dt, kind="ExternalInput"
        ).ap()

    rc_out = nc.dram_tensor("rc_out", [b_shard, D], F32, kind="ExternalOutput").ap()
    ws_out = nc.dram_tensor("ws_out", [1, S], F32, kind="ExternalOutput").ap()
    cs_out = nc.dram_tensor("cs_out", [1, H], F32, kind="ExternalOutput").ap()

    KB = H // P  # 4 contraction blocks for the logits matmul
    SB = S // P  # 2 contraction blocks for the read matmul

    q_src = q_in if PE_BF16 else q_in.bitcast(F32R)
    q_slabs = q_src.rearrange("(n j p) h -> n p j h", p=P, j=slab_tiles)
    c_slabs = c_in.rearrange("(n j p) h -> n p j h", p=P, j=slab_tiles)

    with tile.TileContext(nc) as tc, ExitStack() as ctx:
        consts = ctx.enter_context(tc.tile_pool(name="consts", bufs=1))
        qpool = ctx.enter_context(tc.tile_pool(name="qslab", bufs=2))
        cpool = ctx.enter_context(tc.tile_pool(name="cslab", bufs=2))
        qtpool = ctx.enter_context(tc.tile_pool(name="qt", bufs=2))
        epool = ctx.enter_context(tc.tile_pool(name="exps", bufs=2))
        spool = ctx.enter_context(tc.tile_pool(name="small", bufs=4))
        rcpool = ctx.enter_context(tc.tile_pool(name="rc", bufs=2))
        accpool = ctx.enter_context(tc.tile_pool(name="acc", bufs=1))

        ps_qt = ctx.enter_context(tc.tile_pool(name="ps_qt", bufs=2, space="PSUM"))
        ps_lg = ctx.enter_context(tc.tile_pool(name="ps_lg", bufs=2, space="PSUM"))
        ps_et = ctx.enter_context(tc.tile_pool(name="ps_et", bufs=2, space="PSUM"))
        ps_rc = ctx.enter_context(tc.tile_pool(name="ps_rc", bufs=1, space="PSUM"))
        ps_wa = ctx.enter_context(tc.tile_pool(name="ps_wa", bufs=1, space="PSUM"))

        # ---- constants ----
        wrw_sb = consts.tile([P, KB, 2 * S], pe_dt)
        nc.sync.dma_start(wrw_sb[:], wrw_in.rearrange("(o p) n -> p o n", p=P))
        mem_sb = consts.tile([P, SB, D], pe_dt)
        nc.sync.dma_start(mem_sb[:], mem_in.rearrange("(o p) d -> p o d", p=P))
        ident = consts.tile([P, P], pe_dt)
        make_identity(nc, ident[:])
        ones_col = consts.tile([P, 1], pe_dt)
        nc.vector.memset(ones_col[:], 1.0)
        if use_bias:
            brw_sb = consts.tile([1, 2 * S], pe_dt)
            nc.sync.dma_start(brw_sb[:], brw_in[:])
            ones_row = consts.tile([1, P], pe_dt)
            nc.vector.memset(ones_row[:], 1.0)

        # ---- accumulators ----
        cacc = accpool.tile([P, H], F32)
        nc.vector.memset(cacc[:], 0.0)
        wacc = accpool.tile([1, S], F32)
        nc.vector.memset(wacc[:], 0.0)

        for n in range(n_slabs):
            q_slab = qpool.tile([P, slab_tiles, H], pe_dt, tag="qs")
            if PE_BF16:
                # SWDGE casts fp32 -> bf16 inline during the HBM load.
                nc.gpsimd.dma_start(q_slab[:], q_slabs[n])
            else:
                nc.sync.dma_start(q_slab[:], q_slabs[n])
            c_slab = cpool.tile([P, slab_tiles, H], F32, tag="cs")
            nc.sync.dma_start(c_slab[:], c_slabs[n])

            for j in range(slab_tiles):
                t = n * slab_tiles + j
                q_tile = q_slab[:, j, :]
                c_tile = c_slab[:, j, :]

                # content column-sum accumulation (batch is the partition
                # dim; reduced at the end with a ones-vector matmul)
                nc.vector.tensor_add(cacc[:], cacc[:], c_tile)

                # transpose the query tile: H must be on partitions
                qt_ps = ps_qt.tile([P, H], pe_dt, tag="qt")
                for i in range(KB):
                    nc.tensor.transpose(
                        qt_ps[:, bass.ts(i, P)], q_tile[:, bass.ts(i, P)], ident[:]
                    )
                qt_sb = qtpool.tile([P, H], pe_dt, tag="qtsb")
                nc.vector.tensor_copy(qt_sb[:], qt_ps[:])

                # fused read|write logits
                lg_ps = ps_lg.tile([P, 2 * S], F32, tag="lg")
                for i in range(KB):
                    nc.tensor.matmul(
                        lg_ps[:],
                        qt_sb[:, bass.ts(i, P)],
                        wrw_sb[:, i, :],
                        start=(i == 0),
                        stop=(i == KB - 1) and not use_bias,
                    )
                if use_bias:
                    nc.tensor.matmul(
                        lg_ps[:], ones_row[:], brw_sb[:],
                        start=False, stop=True,
                    )

                # softmax numerators + row sums (no max subtraction needed:
                # |logits| <= ~6 at this problem's scale)
                exp_r = epool.tile([P, S], pe_dt, tag="er")
                rsum = spool.tile([P, 1], F32, tag="rs")
                nc.scalar.activation(
                    exp_r[:], lg_ps[:, :S],
                    mybir.ActivationFunctionType.Exp, accum_out=rsum[:],
                )
                exp_w = epool.tile([P, S], pe_dt, tag="ew")
                wsum = spool.tile([P, 1], F32, tag="ws")
                nc.scalar.activation(
                    exp_w[:], lg_ps[:, S:],
                    mybir.ActivationFunctionType.Exp, accum_out=wsum[:],
                )
                rrec = spool.tile([P, 1], F32, tag="rr")
                nc.vector.reciprocal(rrec[:], rsum[:])
                wrec = spool.tile([P, 1], F32, tag="wr")
                nc.vector.reciprocal(wrec[:], wsum[:])
                wrec_pe = spool.tile([P, 1], pe_dt, tag="wrp")
                nc.vector.tensor_copy(wrec_pe[:], wrec[:])

                # read content: transpose exp_r, contract over S vs memory
                et_ps = ps_et.tile([P, S], pe_dt, tag="et")
                for i in range(SB):
                    nc.tensor.transpose(
                        et_ps[:, bass.ts(i, P)], exp_r[:, bass.ts(i, P)], ident[:]
                    )
                et_sb = qtpool.tile([P, S], pe_dt, tag="etsb")
                nc.vector.tensor_copy(et_sb[:], et_ps[:])

                rc_ps = ps_rc.tile([P, D], F32, tag="rcps")
                for i in range(SB):
                    nc.tensor.matmul(
                        rc_ps[:],
                        et_sb[:, bass.ts(i, P)],
                        mem_sb[:, i, :],
                        start=(i == 0),
                        stop=(i == SB - 1),
                    )
                rc_sb = rcpool.tile([P, D], F32, tag="rcsb")
                nc.vector.tensor_scalar_mul(rc_sb[:], rc_ps[:], rrec[:])
                nc.sync.dma_start(rc_out[bass.ts(t, P), :], rc_sb[:])

                # write-weight partial sum over this tile's batch rows:
                # (1/wsum).T @ exp_w  ->  [1, S]
                wa_ps = ps_wa.tile([1, S], F32, tag="wa")
                nc.tensor.matmul(
                    wa_ps[:], wrec_pe[:], exp_w[:], start=True, stop=True
                )
                nc.vector.tensor_add(wacc[:], wacc[:], wa_ps[:])

        # ---- epilogue: reduce the content accumulator over partitions ----
        cacc_pe = accpool.tile([P, H], pe_dt)
        nc.vector.tensor_copy(cacc_pe[:], cacc[:])
        cs_ps = ps_wa.tile([1, H], F32, tag="wa")
        nc.tensor.matmul(
            cs_ps[:], ones_col[:], cacc_pe[:], start=True, stop=True
        )
        cs_sb = accpool.tile([1, H], F32)
        nc.vector.tensor_copy(cs_sb[:], cs_ps[:])

        nc.sync.dma_start(cs_out[:], cs_sb[:])
        nc.sync.dma_start(ws_out[:], wacc[:])

    nc.compile()
    return nc


_MODULE_CACHE = {}


def _get_module(b_shard, slab_tiles, use_bias):
    key = (b_shard, slab_tiles, use_bias, PE_BF16)
    if key not in _MODULE_CACHE:
        _MODULE_CACHE[key] = build_module(b_shard, slab_tiles, use_bias)
    return _MODULE_CACHE[key]


def kernel(query, content, memory, memory_age,
           W_read, b_read, W_write, b_write, W_content, b_content):
    global LAST_RESULTS, LAST_WALL_NS

    query = np.asarray(query, dtype=np.float32)
    content = np.asarray(content, dtype=np.float32)
    memory = np.asarray(memory, dtype=np.float32)
    memory_age = np.asarray(memory_age, dtype=np.float32)
    W_read = np.asarray(W_read, dtype=np.float32)
    b_read = np.asarray(b_read, dtype=np.float32)
    W_write = np.asarray(W_write, dtype=np.float32)
    b_write = np.asarray(b_write, dtype=np.float32)
    W_content = np.asarray(W_content, dtype=np.float32)
    b_content = np.asarray(b_content, dtype=np.float32)

    b_total, h = query.shape
    assert h == H and b_total % N_CORES == 0
    b_shard = b_total // N_CORES
    use_bias = bool(np.any(b_read) or np.any(b_write))

    nc = _get_module(b_shard, 8 if b_shard % (P * 8) == 0 else 1, use_bias)

    np_pe = ml_dtypes.bfloat16 if PE_BF16 else np.float32
    wrw = np.concatenate([W_read, W_write], axis=1).astype(np_pe)
    mem_d = memory.astype(np_pe)

    in_maps = []
    for c in range(N_CORES):
        sl = slice(c * b_shard, (c + 1) * b_shard)
        m = {
            "q_in": query[sl],
            "c_in": content[sl],
            "wrw_in": wrw,
            "mem_in": mem_d,
        }
        if use_bias:
            m["brw_in"] = np.concatenate([b_read, b_write])[None, :].astype(np_pe)
        in_maps.append(m)

    t0 = time.monotonic_ns()
    res = run_bass_kernel_spmd(nc, in_maps, list(range(N_CORES)))
    LAST_WALL_NS = time.monotonic_ns() - t0
    LAST_RESULTS = res

    read_content = np.concatenate(
        [res.results[c]["rc_out"] for c in range(N_CORES)], axis=0
    )
    w_sum = np.sum([res.results[c]["ws_out"][0] for c in range(N_CORES)], axis=0)
    c_sum = np.sum([res.results[c]["cs_out"][0] for c in range(N_CORES)], axis=0)

    # host finalization (O(S*D)): the cross-core mean + per-slot EMA update
    w_mean = (w_sum / np.float32(b_total)).astype(np.float32)
    c_mean = ((c_sum / np.float32(b_total)) @ W_content + b_content).astype(np.float32)

    active = w_mean > np.float32(0.01)
    consolidation = (
        1.0 / (1.0 + np.exp(-memory_age * np.float32(0.1)))
    ).astype(np.float32)
    alpha = np.where(active, w_mean * consolidation, np.float32(0.0)).astype(
        np.float32
    )[:, None]
    new_memory = ((1.0 - alpha) * memory + alpha * c_mean[None, :]).astype(np.float32)
    new_age = (memory_age + active.astype(np.float32)).astype(np.float32)

    return read_content, new_memory, new_age


# revision 14
# speedup vs baseline: 72037.8926x; 72037.8926x over previous
"""Trainium2 Bass kernel for nn_DenseRecurrentConsciousnessNet.

Computation (B=65536, H=512, S=256, D=64):
    read_weights  = softmax(query @ W_read + b_read)          [B, S]
    read_content  = read_weights @ memory                     [B, D]   (output)
    write_weights = softmax(query @ W_write + b_write)        [B, S]
    w_mean        = write_weights.mean(0)                     [S]
    c_mean        = (content @ W_content + b_content).mean(0) [D]
    alpha         = where(w_mean > 0.01, w_mean * sigmoid(0.1 * age), 0)
    new_memory    = (1 - alpha[:, None]) * memory + alpha[:, None] * c_mean
    new_age       = age + (w_mean > 0.01)                     (outputs)

Sharding: data-parallel over batch across 8 cores.  Each core computes its
read_content shard plus two tiny partial reductions: sum_b(write_weights)
[S] and sum_b(content) [H].  The host sums the 8 partials (the "all-reduce
mean" of the hint) and applies the O(S*D) EMA update.

Per-core device kernel (b_shard = 8192, 64 row tiles of 128):
  - PE transposes each query tile (contraction over H needs H on partitions),
    then one fused matmul against [W_read | W_write] -> logits [128, 512].
  - Softmax without max-subtraction (logits are provably in [-6, 6] for this
    problem scale; exp stays far from fp32 overflow): ACT exp with accum_out
    giving the row sums for free.
  - read: PE-transpose exp_r, matmul against memory, scale rows by 1/rowsum.
  - write: matmul with stationary (1/rowsum_w) [128,1] contracts the batch
    partition dim -> per-tile sum_b(write_weights) [1, 256], accumulated.
  - content: DVE-accumulate tiles, one final ones-vector matmul contracts
    the partition dim -> sum_b(content) [1, 512].
"""

import os
import time
from contextlib import ExitStack

import numpy as np
import ml_dtypes

import concourse.bass as bass
import concourse.bacc as bacc
import concourse.tile as tile
from concourse import mybir
from concourse.bass_utils import run_bass_kernel_spmd
from concourse.masks import make_identity

N_CORES = 8
B, H, S, D = 65536, 512, 256, 64
P = 128

# Precision scheme for the PE input paths.  False: fp32 data flowing through
# the PE as float32r (full-rate for moving dim >= 256).  True: query/weights/
# memory and the exp_r read path are bf16 (transposes and the D=64 matmul run
# at 1 cycle/row instead of 1.5-4).  PSUM accumulation is fp32 either way.
PE_BF16 = True

F32 = mybir.dt.float32
F32R = mybir.dt.float32r
BF16 = mybir.dt.bfloat16

# Stash of the last hardware run, for the local test harness.
LAST_RESULTS = None
LAST_WALL_NS = None


def build_module(b_shard: int, slab_tiles: int, use_bias: bool, repeat: int = 1):
    """Build and compile the per-core Bass module (SPMD: same program on
    every core, per-core data comes from in_maps).

    repeat > 1 wraps the whole body in a device-side For_i loop; used only
    for benchmarking (per-iteration time via K-differencing), outputs are
    still correct since accumulators are reset inside the loop."""
    assert b_shard % (P * slab_tiles) == 0
    n_slabs = b_shard // (P * slab_tiles)
    n_tiles = b_shard // P

    pe_dt = BF16 if PE_BF16 else F32R

    nc = bacc.Bacc(
        "TRN2",
        target_bir_lowering=False,
        debug=False,
        num_devices=N_CORES,
    )

    q_in = nc.dram_tensor("q_in", [b_shard, H], F32, kind="ExternalInput").ap()
    c_in = nc.dram_tensor("c_in", [b_shard, H], F32, kind="ExternalInput").ap()
    wrw_in = nc.dram_tensor("wrw_in", [H, 2 * S], pe_dt, kind="ExternalInput").ap()
    mem_in = nc.dram_tensor("mem_in", [S, D], pe_dt, kind="ExternalInput").ap()
    if use_bias:
        brw_in = nc.dram_tensor(
            "brw_in", [1, 2 * S], pe_dt, kind="ExternalInput"
        ).ap()

    rc_out = nc.dram_tensor("rc_out", [b_shard, D], F32, kind="ExternalOutput").ap()
    ws_out = nc.dram_tensor("ws_out", [1, S], F32, kind="ExternalOutput").ap()
    cs_out = nc.dram_tensor("cs_out", [1, H], F32, kind="ExternalOutput").ap()

    KB = H // P  # 4 contraction blocks for the logits matmul
    SB = S // P  # 2 contraction blocks for the read matmul

    q_src = q_in if PE_BF16 else q_in.bitcast(F32R)
    q_slabs = q_src.rearrange("(n j p) h -> n p j h", p=P, j=slab_tiles)
    c_slabs = c_in.rearrange("(n j p) h -> n p j h", p=P, j=slab_tiles)

    with tile.TileContext(nc) as tc, ExitStack() as ctx:
        consts = ctx.enter_context(tc.tile_pool(name="consts", bufs=1))
        qpool = ctx.enter_context(tc.tile_pool(name="qslab", bufs=2))
        cpool = ctx.enter_context(tc.tile_pool(name="cslab", bufs=2))
        qtpool = ctx.enter_context(tc.tile_pool(name="qt", bufs=2))
        epool = ctx.enter_context(tc.tile_pool(name="exps", bufs=2))
        spool = ctx.enter_context(tc.tile_pool(name="small", bufs=4))
        rcpool = ctx.enter_context(tc.tile_pool(name="rc", bufs=2))
        accpool = ctx.enter_context(tc.tile_pool(name="acc", bufs=1))

        ps_qt = ctx.enter_context(tc.tile_pool(name="ps_qt", bufs=2, space="PSUM"))
        ps_lg = ctx.enter_context(tc.tile_pool(name="ps_lg", bufs=2, space="PSUM"))
        ps_et = ctx.enter_context(tc.tile_pool(name="ps_et", bufs=2, space="PSUM"))
        ps_rc = ctx.enter_context(tc.tile_pool(name="ps_rc", bufs=1, space="PSUM"))
        ps_wa = ctx.enter_context(tc.tile_pool(name="ps_wa", bufs=1, space="PSUM"))

        # ---- constants ----
        wrw_sb = consts.tile([P, KB, 2 * S], pe_dt)
        nc.sync.dma_start(wrw_sb[:], wrw_in.rearrange("(o p) n -> p o n", p=P))
        mem_sb = consts.tile([P, SB, D], pe_dt)
        nc.sync.dma_start(mem_sb[:], mem_in.rearrange("(o p) d -> p o d", p=P))
        ident = consts.tile([P, P], pe_dt)
        make_identity(nc, ident[:])
        ones_col = consts.tile([P, 1], pe_dt)
        nc.vector.memset(ones_col[:], 1.0)
        if use_bias:
            brw_sb = consts.tile([1, 2 * S], pe_dt)
            nc.sync.dma_start(brw_sb[:], brw_in[:])
            ones_row = consts.tile([1, P], pe_dt)
            nc.vector.memset(ones_row[:], 1.0)

        if repeat > 1:
            loop_ctx = tc.For_i(
                0, repeat, 1,
                hint_engines=(
                    mybir.EngineType.PE,
                    mybir.EngineType.DVE,
                    mybir.EngineType.Activation,
                    mybir.EngineType.SP,
                ),
            )
            ctx.enter_context(loop_ctx)

        # ---- accumulators ----
        cacc = accpool.tile([P, H], F32)
        nc.vector.memset(cacc[:], 0.0)
        wacc = accpool.tile([1, S], F32)
        nc.vector.memset(wacc[:], 0.0)

        for n in range(n_slabs):
            q_slab = qpool.tile([P, slab_tiles, H], pe_dt, tag="qs")
            if PE_BF16:
                # SWDGE casts fp32 -> bf16 inline during the HBM load.
                nc.gpsimd.dma_start(q_slab[:], q_slabs[n])
            else:
                nc.sync.dma_start(q_slab[:], q_slabs[n])
            c_slab = cpool.tile([P, slab_tiles, H], F32, tag="cs")
            nc.sync.dma_start(c_slab[:], c_slabs[n])

            for j in range(slab_tiles):
                t = n * slab_tiles + j
                q_tile = q_slab[:, j, :]
                c_tile = c_slab[:, j, :]

                # content column-sum accumulation (batch is the partition
                # dim; reduced at the end with a ones-vector matmul)
                nc.vector.tensor_add(cacc[:], cacc[:], c_tile)

                # transpose the query tile: H must be on partitions
                qt_ps = ps_qt.tile([P, H], pe_dt, tag="qt")
                for i in range(KB):
                    nc.tensor.transpose(
                        qt_ps[:, bass.ts(i, P)], q_tile[:, bass.ts(i, P)], ident[:]
                    )
                qt_sb = qtpool.tile([P, H], pe_dt, tag="qtsb")
                nc.vector.tensor_copy(qt_sb[:], qt_ps[:])

                # fused read|write logits
                lg_ps = ps_lg.tile([P, 2 * S], F32, tag="lg")
                for i in range(KB):
                    nc.tensor.matmul(
                        lg_ps[:],
                        qt_sb[:, bass.ts(i, P)],
                        wrw_sb[:, i, :],
                        start=(i == 0),
                        stop=(i == KB - 1) and not use_bias,
                    )
                if use_bias:
                    nc.tensor.matmul(
                        lg_ps[:], ones_row[:], brw_sb[:],
                        start=False, stop=True,
                    )

                # softmax numerators + row sums (no max subtraction needed:
                # |logits| <= ~6 at this problem's scale)
                exp_r = epool.tile([P, S], pe_dt, tag="er")
                rsum = spool.tile([P, 1], F32, tag="rs")
                nc.scalar.activation(
                    exp_r[:], lg_ps[:, :S],
                    mybir.ActivationFunctionType.Exp, accum_out=rsum[:],
                )
                exp_w = epool.tile([P, S], pe_dt, tag="ew")
                wsum = spool.tile([P, 1], F32, tag="ws")
                nc.scalar.activation(
                    exp_w[:], lg_ps[:, S:],
                    mybir.ActivationFunctionType.Exp, accum_out=wsum[:],
                )
                rrec = spool.tile([P, 1], F32, tag="rr")
                nc.vector.reciprocal(rrec[:], rsum[:])
                wrec = spool.tile([P, 1], F32, tag="wr")
                nc.vector.reciprocal(wrec[:], wsum[:])
                wrec_pe = spool.tile([P, 1], pe_dt, tag="wrp")
                nc.vector.tensor_copy(wrec_pe[:], wrec[:])

                # read content: transpose exp_r, contract over S vs memory
                et_ps = ps_et.tile([P, S], pe_dt, tag="et")
                for i in range(SB):
                    nc.tensor.transpose(
                        et_ps[:, bass.ts(i, P)], exp_r[:, bass.ts(i, P)], ident[:]
                    )
                et_sb = qtpool.tile([P, S], pe_dt, tag="etsb")
                nc.vector.tensor_copy(et_sb[:], et_ps[:])

                rc_ps = ps_rc.tile([P, D], F32, tag="rcps")
                for i in range(SB):
                    nc.tensor.matmul(
                        rc_ps[:],
                        et_sb[:, bass.ts(i, P)],
                        mem_sb[:, i, :],
                        start=(i == 0),
                        stop=(i == SB - 1),
                    )
                rc_sb = rcpool.tile([P, D], F32, tag="rcsb")
                nc.vector.tensor_scalar_mul(rc_sb[:], rc_ps[:], rrec[:])
                nc.sync.dma_start(rc_out[bass.ts(t, P), :], rc_sb[:])

                # write-weight partial sum over this tile's batch rows:
                # (1/wsum).T @ exp_w  ->  [1, S]
                wa_ps = ps_wa.tile([1, S], F32, tag="wa")
                nc.tensor.matmul(
                    wa_ps[:], wrec_pe[:], exp_w[:], start=True, stop=True
                )
                nc.vector.tensor_add(wacc[:], wacc[:], wa_ps[:])

        # ---- epilogue: reduce the content accumulator over partitions ----
        cacc_pe = accpool.tile([P, H], pe_dt)
        nc.vector.tensor_copy(cacc_pe[:], cacc[:])
        cs_ps = ps_wa.tile([1, H], F32, tag="wa")
        nc.tensor.matmul(
            cs_ps[:], ones_col[:], cacc_pe[:], start=True, stop=True
        )
        cs_sb = accpool.tile([1, H], F32)
        nc.vector.tensor_copy(cs_sb[:], cs_ps[:])

        nc.sync.dma_start(cs_out[:], cs_sb[:])
        nc.sync.dma_start(ws_out[:], wacc[:])

    nc.compile()
    return nc


_MODULE_CACHE = {}


def _get_module(b_shard, slab_tiles, use_bias):
    key = (b_shard, slab_tiles, use_bias, PE_BF16)
    if key not in _MODULE_CACHE:
        _MODULE_CACHE[key] = build_module(b_shard, slab_tiles, use_bias)
    return _MODULE_CACHE[key]


def kernel(query, content, memory, memory_age,
           W_read, b_read, W_write, b_write, W_content, b_content):
    global LAST_RESULTS, LAST_WALL_NS

    query = np.asarray(query, dtype=np.float32)
    content = np.asarray(content, dtype=np.float32)
    memory = np.asarray(memory, dtype=np.float32)
    memory_age = np.asarray(memory_age, dtype=np.float32)
    W_read = np.asarray(W_read, dtype=np.float32)
    b_read = np.asarray(b_read, dtype=np.float32)
    W_write = np.asarray(W_write, dtype=np.float32)
    b_write = np.asarray(b_write, dtype=np.float32)
    W_content = np.asarray(W_content, dtype=np.float32)
    b_content = np.asarray(b_content, dtype=np.float32)

    b_total, h = query.shape
    assert h == H and b_total % N_CORES == 0
    b_shard = b_total // N_CORES
    use_bias = bool(np.any(b_read) or np.any(b_write))

    nc = _get_module(b_shard, 8 if b_shard % (P * 8) == 0 else 1, use_bias)

    np_pe = ml_dtypes.bfloat16 if PE_BF16 else np.float32
    wrw = np.concatenate([W_read, W_write], axis=1).astype(np_pe)
    mem_d = memory.astype(np_pe)

    in_maps = []
    for c in range(N_CORES):
        sl = slice(c * b_shard, (c + 1) * b_shard)
        m = {
            "q_in": query[sl],
            "c_in": content[sl],
            "wrw_in": wrw,
            "mem_in": mem_d,
        }
        if use_bias:
            m["brw_in"] = np.concatenate([b_read, b_write])[None, :].astype(np_pe)
        in_maps.append(m)

    t0 = time.monotonic_ns()
    res = run_bass_kernel_spmd(nc, in_maps, list(range(N_CORES)))
    LAST_WALL_NS = time.monotonic_ns() - t0
    LAST_RESULTS = res

    read_content = np.concatenate(
        [res.results[c]["rc_out"] for c in range(N_CORES)], axis=0
    )
    w_sum = np.sum([res.results[c]["ws_out"][0] for c in range(N_CORES)], axis=0)
    c_sum = np.sum([res.results[c]["cs_out"][0] for c in range(N_CORES)], axis=0)

    # host finalization (O(S*D)): the cross-core mean + per-slot EMA update
    w_mean = (w_sum / np.float32(b_total)).astype(np.float32)
    c_mean = ((c_sum / np.float32(b_total)) @ W_content + b_content).astype(np.float32)

    active = w_mean > np.float32(0.01)
    consolidation = (
        1.0 / (1.0 + np.exp(-memory_age * np.float32(0.1)))
    ).astype(np.float32)
    alpha = np.where(active, w_mean * consolidation, np.float32(0.0)).astype(
        np.float32
    )[:, None]
    new_memory = ((1.0 - alpha) * memory + alpha * c_mean[None, :]).astype(np.float32)
    new_age = (memory_age + active.astype(np.float32)).astype(np.float32)

    return read_content, new_memory, new_age
